# revision 4
# baseline (speedup 1.0000x reference)
"""Paged-attention decode (vLLM-style) for Trainium2, 8 NeuronCores.

Sharding: tensor-parallel over KV heads. Core h owns KV head h and query
heads 4h..4h+3. block_tables / seq_lens are host-visible integers, so the
device program is fully static: gather addresses, masking boundaries and
loop trip counts are baked into the instruction stream at build time.

Host prep (not on the device clock):
  - apply the 16-row new-token scatter to a host copy of the cache
    (exactly reference step 1),
  - lay out K as K^T [D=128, 65536] per head  -> QK matmul stationary tiles,
  - lay out V as [p=128, chunk=512, d=128] per head (position = chunk*128+p)
    -> PV matmul stationary tiles,
  - q as qT [128, 64] (col = 4*b+g).

Device per sequence b (length L, C = ceil(L/128) chunks):
  scores[s,g]  : per chunk c: matmul(out=scores[:,4c:4c+4],
                 lhsT=K^T chunk [128,128], rhs=qT[:,4b:4b+4])      (PSUM)
  probs        : ACT exp(scale*scores) -> SBUF [128, 4C]; zero the padding
  out[d,g]     : per chunk c: matmul(acc, lhsT=V chunk [128(s),128(d)],
                 rhs=probs[:,4c:4c+4], accumulate)                 (PSUM)
  denom        : matmul(lhsT=ones[128,1], rhs=probs[:, :4C]) -> [1,4C],
                 DVE strided reduce -> [1,4], DVE reciprocal -> r [1,4]
  broadcast    : matmul(lhsT=ones[1,128], rhs=r[1,4]) -> [128,4]  (PSUM)
  normalize    : DVE mul (acc * r_bcast) -> SBUF [128,4]
  transpose    : PE transpose -> [4,128], copy to SBUF, DMA to out[b]
"""

import math
import os
import sys
import tempfile

import numpy as np

for _p in ("/opt/trn_rl_repo", "/opt/pypackages"):
    if os.path.isdir(_p) and _p not in sys.path:
        sys.path.append(_p)

B = 16
H = 32
HKV = 8
D = 128
G = H // HKV  # 4 query heads per kv head
BLOCK = 16
SLOTS = 65536  # total cache slots (NUM_BLOCKS * BLOCK)
VCH = SLOTS // 128  # 512 V chunks in the cache
SCALE = 1.0 / math.sqrt(D)
N_CORES = 8

# K/V slab DMAs are split into pieces of this many positions so compute can
# start before a whole sequence has landed (512 KiB per piece).
DMA_SPLIT = 1024

TRACE = False
TRACE_ALL_CORES = False
LAST_EXEC_NS = None
LAST_RESULTS = None

_CACHE = {}


def _coalesced_runs(bt_row, L):
    """[(dst_pos, src_slot, n)] covering positions [0, L), merged when the
    source slots are contiguous (always, for arange block tables)."""
    runs = []
    nblk = (L + BLOCK - 1) // BLOCK
    for i in range(nblk):
        s = int(bt_row[i]) * BLOCK
        a = i * BLOCK
        n = min(BLOCK, L - a)
        if runs and runs[-1][1] + runs[-1][2] == s:
            runs[-1][2] += n
        else:
            runs.append([a, s, n])
    return [tuple(r) for r in runs]


def _v_rects(a, s, n):
    """Decompose a (dst_pos=a, src_slot=s, len=n) run into rectangles for the
    chunked V layout [p, c, d] with pos = c*128 + p. Yields
    (pd, ps, m, cd, cs, k): dst partitions [pd, pd+m) chunks [cd, cd+k),
    src partitions [ps, ps+m) chunks [cs, cs+k)."""
    rects = []
    if (s - a) % 128 == 0:
        dc = (s - a) // 128
        x = a
        end = a + n
        # partial head up to the next 128 boundary
        if x % 128:
            m = min(128 - x % 128, end - x)
            rects.append((x % 128, x % 128, m, x // 128, x // 128 + dc, 1))
            x += m
        # full-chunk middle
        k = (end - x) // 128
        if k > 0:
            rects.append((0, 0, 128, x // 128, x // 128 + dc, k))
            x += k * 128
        if x < end:
            rects.append((0, 0, end - x, x // 128, x // 128 + dc, 1))
    else:
        # misaligned: split at every dst and src 128-boundary
        x = a
        end = a + n
        while x < end:
            y = x - a + s
            m = min(128 - x % 128, 128 - y % 128, end - x)
            rects.append((x % 128, y % 128, m, x // 128, y // 128, 1))
            x += m
    return rects


def _build(seq_lens, runs_key):
    import concourse.bass as bass  # noqa: F401
    import concourse.mybir as mybir
    import concourse.tile as tile
    from concourse import bacc
    from concourse.masks import make_identity

    f32 = mybir.dt.float32
    Exp = mybir.ActivationFunctionType.Exp

    runs_per_seq = {b: runs for b, runs in runs_key}

    nc = bacc.Bacc(
        "TRN2", target_bir_lowering=False, debug=False, num_devices=N_CORES
    )
    kt = nc.dram_tensor("kt", [128, SLOTS], f32, kind="ExternalInput").ap()
    vc = nc.dram_tensor("vc", [128, VCH, 128], f32, kind="ExternalInput").ap()
    qt = nc.dram_tensor("qt", [128, B * G], f32, kind="ExternalInput").ap()
    outd = nc.dram_tensor("out", [B, G * 128], f32, kind="ExternalOutput").ap()
    out3 = outd.rearrange("b (g d) -> b g d", g=G)

    order = sorted(range(B), key=lambda b: -int(seq_lens[b]))

    with tile.TileContext(nc) as tc:
        with (
            tc.tile_pool(name="const", bufs=1) as const,
            tc.tile_pool(name="big", bufs=3) as big,
            tc.tile_pool(name="small", bufs=3) as small,
            tc.tile_pool(name="ps_sc", bufs=2, space="PSUM") as ps_sc,
            tc.tile_pool(name="ps_epi", bufs=2, space="PSUM") as ps_epi,
        ):
            qt_sb = const.tile([128, B * G], f32)
            nc.sync.dma_start(out=qt_sb, in_=qt)
            ones_col = const.tile([128, 1], f32)
            nc.vector.memset(ones_col, 1.0)
            ones_row = const.tile([1, 128], f32)
            nc.vector.memset(ones_row, 1.0)
            ident = const.tile([128, 128], f32)
            make_identity(nc, ident)

            for b in order:
                L = int(seq_lens[b])
                C = (L + 127) // 128
                runs = runs_per_seq[b]

                ktile = big.tile([128, C * 128], f32, tag="ktile", name=f"kt{b}")
                vtile = big.tile([128, C, 128], f32, tag="vtile", name=f"vt{b}")

                if L % 128:
                    # engine ops need 32-aligned partition bases, so zero the
                    # whole trailing chunk first; the DMA then fills the valid
                    # rows (Tile orders the overlapping writes).
                    nc.vector.memset(vtile[:, C - 1, :], 0.0)
                for (a, s, n) in runs:
                    for off in range(0, n, DMA_SPLIT):
                        m = min(DMA_SPLIT, n - off)
                        nc.sync.dma_start(
                            out=ktile[:, a + off : a + off + m],
                            in_=kt[:, s + off : s + off + m],
                        )
                    for (pd, ps, m, cd, cs, k) in _v_rects(a, s, n):
                        for coff in range(0, k, DMA_SPLIT // 128):
                            kk = min(DMA_SPLIT // 128, k - coff)
                            nc.sync.dma_start(
                                out=vtile[pd : pd + m, cd + coff : cd + coff + kk, :],
                                in_=vc[ps : ps + m, cs + coff : cs + coff + kk, :],
                            )
                if L < C * 128:
                    nc.vector.memset(ktile[:, L : C * 128], 0.0)

                scores = ps_sc.tile([128, 4 * C], f32, tag="scores", name=f"sc{b}")
                for c in range(C):
                    nc.tensor.matmul(
                        scores[:, 4 * c : 4 * c + 4],
                        lhsT=ktile[:, c * 128 : (c + 1) * 128],
                        rhs=qt_sb[:, G * b : G * b + G],
                        start=True,
                        stop=True,
                    )

                probs = small.tile([128, 4 * C], f32, tag="probs", name=f"pr{b}")
                if L % 128:
                    nc.vector.memset(probs[:, 4 * (C - 1) : 4 * C], 0.0)
                    if C > 1:
                        nc.scalar.activation(
                            probs[:, : 4 * (C - 1)],
                            scores[:, : 4 * (C - 1)],
                            Exp,
                            scale=SCALE,
                        )
                    nc.scalar.activation(
                        probs[0 : L % 128, 4 * (C - 1) : 4 * C],
                        scores[0 : L % 128, 4 * (C - 1) : 4 * C],
                        Exp,
                        scale=SCALE,
                    )
                else:
                    nc.scalar.activation(probs, scores, Exp, scale=SCALE)

                # epi1: cols 0:4 = PV accumulator [128(d), 4(g)];
                #       cols 8:8+4C (partition 0) = per-chunk prob sums
                epi1 = ps_epi.tile([128, 136], f32, tag="epi1", name=f"e1{b}")
                for c in range(C):
                    nc.tensor.matmul(
                        epi1[:, 0:4],
                        lhsT=vtile[:, c, :],
                        rhs=probs[:, 4 * c : 4 * c + 4],
                        start=(c == 0),
                        stop=(c == C - 1),
                    )
                nc.tensor.matmul(
                    epi1[0:1, 8 : 8 + 4 * C],
                    lhsT=ones_col,
                    rhs=probs,
                    start=True,
                    stop=True,
                )

                den4 = small.tile([1, 4], f32, tag="den4", name=f"d4{b}")
                nc.vector.reduce_sum(
                    out=den4,
                    in_=epi1[0:1, 8 : 8 + 4 * C].rearrange("p (c g) -> p g c", g=4),
                    axis=mybir.AxisListType.X,
                )
                r4 = small.tile([1, 4], f32, tag="r4", name=f"r4{b}")
                nc.vector.reciprocal(r4, den4)

                # epi2: cols 0:4 = broadcast of r4 to 128 partitions;
                #       cols 4:132 (partitions 0:4) = transposed output
                epi2 = ps_epi.tile([128, 132], f32, tag="epi2", name=f"e2{b}")
                nc.tensor.matmul(
                    epi2[:, 0:4], lhsT=ones_row, rhs=r4, start=True, stop=True
                )
                r_sb = small.tile([128, 4], f32, tag="r_sb", name=f"rb{b}")
                nc.vector.tensor_copy(r_sb, epi2[:, 0:4])
                o_sb = small.tile([128, 4], f32, tag="o_sb", name=f"ob{b}")
                nc.vector.tensor_mul(o_sb, epi1[:, 0:4], r_sb)

                nc.tensor.transpose(epi2[0:4, 4:132], o_sb, ident)
                o_fin = small.tile([4, 128], f32, tag="o_fin", name=f"of{b}")
                nc.vector.tensor_copy(o_fin, epi2[0:4, 4:132])
                nc.scalar.dma_start(out=out3[b], in_=o_fin)

    nc.compile()
    return nc


def kernel(query, key, value, kv_cache, block_tables, seq_lens, slot_mapping):
    global LAST_EXEC_NS, LAST_RESULTS
    from concourse import bass_utils

    query = np.asarray(query, dtype=np.float32)
    key = np.asarray(key, dtype=np.float32)
    value = np.asarray(value, dtype=np.float32)
    kv_cache = np.asarray(kv_cache, dtype=np.float32)
    block_tables = np.asarray(block_tables)
    seq_lens = np.asarray(seq_lens)
    slot_mapping = np.asarray(slot_mapping)

    # --- host prep: apply new-token scatter (reference step 1) ---
    kc = np.array(kv_cache[0].reshape(SLOTS, HKV, D))
    vcn = np.array(kv_cache[1].reshape(SLOTS, HKV, D))
    kc[slot_mapping] = key.reshape(B, HKV, D)
    vcn[slot_mapping] = value.reshape(B, HKV, D)

    # per-head device layouts
    ktT = np.ascontiguousarray(kc.transpose(1, 2, 0))  # [8, 128(d), 65536]
    vch = np.ascontiguousarray(
        vcn.reshape(VCH, 128, HKV, D).transpose(2, 1, 0, 3)
    )  # [8, 128(p), 512(c), 128(d)]
    qtA = np.ascontiguousarray(
        query.reshape(B, HKV, G, D).transpose(1, 3, 0, 2)
    ).reshape(HKV, D, B * G)  # [8, 128(d), 64]

    lens = [int(x) for x in seq_lens]
    runs_key = tuple(
        (b, tuple(_coalesced_runs(block_tables[b], max(lens[b], 1))))
        for b in range(B)
    )
    cache_key = (tuple(lens), runs_key)
    if cache_key not in _CACHE:
        _CACHE[cache_key] = _build(lens, runs_key)
    nc = _CACHE[cache_key]

    in_maps = [
        {"kt": ktT[h], "vc": vch[h], "qt": np.ascontiguousarray(qtA[h])}
        for h in range(N_CORES)
    ]
    kwargs = {}
    if TRACE:
        kwargs["trace"] = True
        kwargs["tmpdir"] = tempfile.mkdtemp(prefix="bass_attn_")
        if TRACE_ALL_CORES:
            kwargs["trace_cores"] = list(range(N_CORES))
    res = bass_utils.run_bass_kernel_spmd(
        nc, in_maps, list(range(N_CORES)), **kwargs
    )
    LAST_EXEC_NS = res.exec_time_ns
    LAST_RESULTS = res

    out = np.empty((B, H * D), dtype=np.float32)
    for h in range(N_CORES):
        out[:, h * G * 128 : (h + 1) * G * 128] = res.results[h]["out"]
    return out


# revision 5
# speedup vs baseline: 1.6571x; 1.6571x over previous
"""Paged-attention decode (vLLM-style) for Trainium2, 8 NeuronCores.

Sharding: tensor-parallel over KV heads. Core h owns KV head h and query
heads 4h..4h+3. block_tables / seq_lens are host-visible integers, so the
device program is fully static: gather addresses, masking boundaries and
loop trip counts are baked into the instruction stream at build time.

Precision strategy: TRN2's fp32 matmul is lowered by the compiler into two
half-rate passes with a ~210ns weight reload each, which made the fp32
version PE-bound (~430ns per logical matmul regardless of N). Instead we
split every fp32 operand x into bf16 hi + bf16 lo (x ~= hi + lo, the same
decomposition the hardware fp32 path uses), ship both halves from the host
(same total bytes as fp32), and run bf16 matmuls which get the fast weight
load path (~53ns per 128-col load). Products keep the hi*hi, hi*lo and
lo*hi terms (~2^-17 relative error, matching hardware fp32 matmul quality).
To avoid reloading the hi weights for the hi*lo term, the moving operand is
the concatenation [x_hi | x_lo]; the hi*lo product lands in spill columns
of PSUM and is folded in afterwards with one vector add per sequence.

Host prep (not on the device clock):
  - apply the 16-row new-token scatter to a host copy of the cache
    (exactly reference step 1),
  - K as K^T [D=128, 65536] hi/lo bf16 per head -> QK stationary tiles,
  - V as [p=128, chunk=512, d=128] hi/lo bf16 per head (pos = chunk*128+p)
    -> PV stationary tiles,
  - q as [128(d), 2(hi/lo), 16(b), 4(g)] bf16.

Device per sequence b (length L, C = ceil(L/128) chunks):
  scores   : per chunk c: matmul(psum[:, 8c:8c+8], lhsT=Khi_c, rhs=q_cat)
             + matmul(psum[:, 8c:8c+4], lhsT=Klo_c, rhs=q_hi, accumulate)
  probs    : DVE fold spill cols, ACT exp(scale*x), zero the padding tail
  p_cat    : DVE split probs into interleaved bf16 hi/lo [128, C, 2, 4]
  out[d,g] : per chunk c: matmul(acc[:, 0:8], lhsT=Vhi_c, rhs=p_cat_c)
             + matmul(acc[:, 0:4], lhsT=Vlo_c, rhs=p_hi_c), accumulated
  denom    : matmul(lhsT=ones[128,1] f32, rhs=probs f32) -> [1,4C],
             DVE strided reduce -> [1,4], DVE reciprocal -> r [1,4]
  bcast    : matmul(lhsT=ones[1,128], rhs=r) -> [128,4] (PSUM)
  final    : DVE fold PV spill, mul by r_bcast, PE transpose -> [4,128],
             copy to SBUF, DMA to out[b]
"""

import math
import os
import sys
import tempfile

import numpy as np

for _p in ("/opt/trn_rl_repo", "/opt/pypackages"):
    if os.path.isdir(_p) and _p not in sys.path:
        sys.path.append(_p)

import ml_dtypes

BF16 = ml_dtypes.bfloat16

B = 16
H = 32
HKV = 8
D = 128
G = H // HKV  # 4 query heads per kv head
BLOCK = 16
SLOTS = 65536  # total cache slots (NUM_BLOCKS * BLOCK)
VCH = SLOTS // 128  # 512 V chunks in the cache
SCALE = 1.0 / math.sqrt(D)
N_CORES = 8

# K/V slab DMAs are split into pieces of this many positions so compute can
# start before a whole sequence has landed.
DMA_SPLIT = 1024

TRACE = False
TRACE_ALL_CORES = False
LAST_EXEC_NS = None
LAST_RESULTS = None

_CACHE = {}


def _coalesced_runs(bt_row, L):
    """[(dst_pos, src_slot, n)] covering positions [0, L), merged when the
    source slots are contiguous (always, for arange block tables)."""
    runs = []
    nblk = (L + BLOCK - 1) // BLOCK
    for i in range(nblk):
        s = int(bt_row[i]) * BLOCK
        a = i * BLOCK
        n = min(BLOCK, L - a)
        if runs and runs[-1][1] + runs[-1][2] == s:
            runs[-1][2] += n
        else:
            runs.append([a, s, n])
    return [tuple(r) for r in runs]


def _v_rects(a, s, n):
    """Decompose a (dst_pos=a, src_slot=s, len=n) run into rectangles for the
    chunked V layout [p, c, d] with pos = c*128 + p. Yields
    (pd, ps, m, cd, cs, k): dst partitions [pd, pd+m) chunks [cd, cd+k),
    src partitions [ps, ps+m) chunks [cs, cs+k)."""
    rects = []
    if (s - a) % 128 == 0:
        dc = (s - a) // 128
        x = a
        end = a + n
        if x % 128:
            m = min(128 - x % 128, end - x)
            rects.append((x % 128, x % 128, m, x // 128, x // 128 + dc, 1))
            x += m
        k = (end - x) // 128
        if k > 0:
            rects.append((0, 0, 128, x // 128, x // 128 + dc, k))
            x += k * 128
        if x < end:
            rects.append((0, 0, end - x, x // 128, x // 128 + dc, 1))
    else:
        x = a
        end = a + n
        while x < end:
            y = x - a + s
            m = min(128 - x % 128, 128 - y % 128, end - x)
            rects.append((x % 128, y % 128, m, x // 128, y // 128, 1))
            x += m
    return rects


def _build(seq_lens, runs_key):
    import concourse.bass as bass  # noqa: F401
    import concourse.mybir as mybir
    import concourse.tile as tile
    from concourse import bacc
    from concourse.masks import make_identity

    f32 = mybir.dt.float32
    bf16 = mybir.dt.bfloat16
    Exp = mybir.ActivationFunctionType.Exp

    runs_per_seq = {b: runs for b, runs in runs_key}

    nc = bacc.Bacc(
        "TRN2", target_bir_lowering=False, debug=False, num_devices=N_CORES
    )
    khi_d = nc.dram_tensor("khi", [128, SLOTS], bf16, kind="ExternalInput").ap()
    klo_d = nc.dram_tensor("klo", [128, SLOTS], bf16, kind="ExternalInput").ap()
    vhi_d = nc.dram_tensor("vhi", [128, VCH, 128], bf16, kind="ExternalInput").ap()
    vlo_d = nc.dram_tensor("vlo", [128, VCH, 128], bf16, kind="ExternalInput").ap()
    qc_d = nc.dram_tensor("qc", [128, 2, B, G], bf16, kind="ExternalInput").ap()
    outd = nc.dram_tensor("out", [B, G * 128], f32, kind="ExternalOutput").ap()
    out3 = outd.rearrange("b (g d) -> b g d", g=G)

    order = sorted(range(B), key=lambda b: -int(seq_lens[b]))

    with tile.TileContext(nc) as tc:
        with (
            tc.tile_pool(name="const", bufs=1) as const,
            tc.tile_pool(name="big", bufs=3) as big,
            tc.tile_pool(name="small", bufs=3) as small,
            tc.tile_pool(name="ps_sc", bufs=2, space="PSUM") as ps_sc,
            tc.tile_pool(name="ps_epi", bufs=2, space="PSUM") as ps_epi,
        ):
            qc_sb = const.tile([128, 2, B, G], bf16)
            nc.sync.dma_start(out=qc_sb, in_=qc_d)
            ones_col = const.tile([128, 1], f32)
            nc.vector.memset(ones_col, 1.0)
            ones_row = const.tile([1, 128], f32)
            nc.vector.memset(ones_row, 1.0)
            ident = const.tile([128, 128], f32)
            make_identity(nc, ident)

            for b in order:
                L = int(seq_lens[b])
                C = (L + 127) // 128
                runs = runs_per_seq[b]

                khi = big.tile([128, C * 128], bf16, tag="khi", name=f"kh{b}")
                klo = big.tile([128, C * 128], bf16, tag="klo", name=f"kl{b}")
                vhi = big.tile([128, C, 128], bf16, tag="vhi", name=f"vh{b}")
                vlo = big.tile([128, C, 128], bf16, tag="vlo", name=f"vl{b}")

                if L % 128:
                    # engine ops need 32-aligned partition bases, so zero the
                    # whole trailing chunk first; the DMAs then fill the valid
                    # rows (Tile orders the overlapping writes).
                    nc.vector.memset(vhi[:, C - 1, :], 0.0)
                    nc.vector.memset(vlo[:, C - 1, :], 0.0)
                for (a, s, n) in runs:
                    for off in range(0, n, DMA_SPLIT):
                        m = min(DMA_SPLIT, n - off)
                        nc.sync.dma_start(
                            out=khi[:, a + off : a + off + m],
                            in_=khi_d[:, s + off : s + off + m],
                        )
                        nc.sync.dma_start(
                            out=klo[:, a + off : a + off + m],
                            in_=klo_d[:, s + off : s + off + m],
                        )
                    for (pd, ps, m, cd, cs, k) in _v_rects(a, s, n):
                        for coff in range(0, k, DMA_SPLIT // 128):
                            kk = min(DMA_SPLIT // 128, k - coff)
                            nc.sync.dma_start(
                                out=vhi[pd : pd + m, cd + coff : cd + coff + kk, :],
                                in_=vhi_d[ps : ps + m, cs + coff : cs + coff + kk, :],
                            )
                            nc.sync.dma_start(
                                out=vlo[pd : pd + m, cd + coff : cd + coff + kk, :],
                                in_=vlo_d[ps : ps + m, cs + coff : cs + coff + kk, :],
                            )
                if L < C * 128:
                    nc.vector.memset(khi[:, L : C * 128], 0.0)
                    nc.vector.memset(klo[:, L : C * 128], 0.0)

                # QK: psum cols per chunk: [8c, 8c+4) = hi*q_hi + lo*q_hi,
                #     [8c+4, 8c+8) = hi*q_lo spill
                scores = ps_sc.tile([128, 8 * C], f32, tag="scores", name=f"sc{b}")
                for c in range(C):
                    nc.tensor.matmul(
                        scores[:, 8 * c : 8 * c + 8],
                        lhsT=khi[:, c * 128 : (c + 1) * 128],
                        rhs=qc_sb[:, :, b, :],
                        start=True,
                        stop=False,
                        skip_group_check=True,
                    )
                    nc.tensor.matmul(
                        scores[:, 8 * c : 8 * c + 4],
                        lhsT=klo[:, c * 128 : (c + 1) * 128],
                        rhs=qc_sb[:, 0, b, :],
                        start=False,
                        stop=True,
                        skip_group_check=True,
                    )

                sc4 = scores.rearrange("p (c j g) -> p c j g", j=2, g=G)
                spill_s = small.tile([128, C, G], f32, tag="spill_s", name=f"ss{b}")
                nc.vector.tensor_copy(spill_s, sc4[:, :, 1, :])
                probs_f = small.tile([128, C, G], f32, tag="probs_f", name=f"pf{b}")
                nc.vector.tensor_add(probs_f, sc4[:, :, 0, :], spill_s)

                probs_e = small.tile([128, C * G], f32, tag="probs_e", name=f"pe{b}")
                pf2 = probs_f.rearrange("p c g -> p (c g)")
                if L % 128:
                    nc.vector.memset(probs_e[:, G * (C - 1) : G * C], 0.0)
                    if C > 1:
                        nc.scalar.activation(
                            probs_e[:, : G * (C - 1)],
                            pf2[:, : G * (C - 1)],
                            Exp,
                            scale=SCALE,
                        )
                    nc.scalar.activation(
                        probs_e[0 : L % 128, G * (C - 1) : G * C],
                        pf2[0 : L % 128, G * (C - 1) : G * C],
                        Exp,
                        scale=SCALE,
                    )
                else:
                    nc.scalar.activation(probs_e, pf2, Exp, scale=SCALE)

                # interleaved bf16 hi/lo of probs: [128, C, 2, G]
                pe3 = probs_e.rearrange("p (c g) -> p c g", g=G)
                pcat = small.tile([128, C, 2, G], bf16, tag="pcat", name=f"pc{b}")
                nc.vector.tensor_copy(pcat[:, :, 0, :], pe3)
                nc.vector.tensor_sub(pcat[:, :, 1, :], pe3, pcat[:, :, 0, :])

                # epi1: cols 0:4 = PV main, 4:8 = PV hi*p_lo spill,
                #       cols 8:8+4C (partition 0) = per-chunk prob sums
                epi1 = ps_epi.tile([128, 136], f32, tag="epi1", name=f"e1{b}")
                for c in range(C):
                    nc.tensor.matmul(
                        epi1[:, 0:8],
                        lhsT=vhi[:, c, :],
                        rhs=pcat[:, c, :, :],
                        start=(c == 0),
                        stop=False,
                        skip_group_check=True,
                    )
                    nc.tensor.matmul(
                        epi1[:, 0:4],
                        lhsT=vlo[:, c, :],
                        rhs=pcat[:, c, 0, :],
                        start=False,
                        stop=(c == C - 1),
                        skip_group_check=True,
                    )
                nc.tensor.matmul(
                    epi1[0:1, 8 : 8 + G * C],
                    lhsT=ones_col,
                    rhs=probs_e,
                    start=True,
                    stop=True,
                )

                den4 = small.tile([1, G], f32, tag="den4", name=f"d4{b}")
                nc.vector.reduce_sum(
                    out=den4,
                    in_=epi1[0:1, 8 : 8 + G * C].rearrange("p (c g) -> p g c", g=G),
                    axis=mybir.AxisListType.X,
                )
                r4 = small.tile([1, G], f32, tag="r4", name=f"r4{b}")
                nc.vector.reciprocal(r4, den4)

                # epi2: cols 0:4 = broadcast of r4; cols 4:132 = transposed out
                epi2 = ps_epi.tile([128, 132], f32, tag="epi2", name=f"e2{b}")
                nc.tensor.matmul(
                    epi2[:, 0:4], lhsT=ones_row, rhs=r4, start=True, stop=True
                )
                r_sb = small.tile([128, G], f32, tag="r_sb", name=f"rb{b}")
                nc.vector.tensor_copy(r_sb, epi2[:, 0:4])

                pv_sp = small.tile([128, G], f32, tag="pv_sp", name=f"pv{b}")
                nc.vector.tensor_copy(pv_sp, epi1[:, 4:8])
                acc_sb = small.tile([128, G], f32, tag="acc_sb", name=f"ac{b}")
                nc.vector.tensor_add(acc_sb, epi1[:, 0:4], pv_sp)
                o_sb = small.tile([128, G], f32, tag="o_sb", name=f"ob{b}")
                nc.vector.tensor_mul(o_sb, acc_sb, r_sb)

                nc.tensor.transpose(epi2[0:4, 4:132], o_sb, ident)
                o_fin = small.tile([G, 128], f32, tag="o_fin", name=f"of{b}")
                nc.vector.tensor_copy(o_fin, epi2[0:4, 4:132])
                nc.scalar.dma_start(out=out3[b], in_=o_fin)

    nc.compile()
    return nc


def _hi_lo(x):
    hi = x.astype(BF16)
    lo = (x - hi.astype(np.float32)).astype(BF16)
    return hi, lo


def kernel(query, key, value, kv_cache, block_tables, seq_lens, slot_mapping):
    global LAST_EXEC_NS, LAST_RESULTS
    from concourse import bass_utils

    query = np.asarray(query, dtype=np.float32)
    key = np.asarray(key, dtype=np.float32)
    value = np.asarray(value, dtype=np.float32)
    kv_cache = np.asarray(kv_cache, dtype=np.float32)
    block_tables = np.asarray(block_tables)
    seq_lens = np.asarray(seq_lens)
    slot_mapping = np.asarray(slot_mapping)

    # --- host prep: apply new-token scatter (reference step 1) ---
    kc = np.array(kv_cache[0].reshape(SLOTS, HKV, D))
    vcn = np.array(kv_cache[1].reshape(SLOTS, HKV, D))
    kc[slot_mapping] = key.reshape(B, HKV, D)
    vcn[slot_mapping] = value.reshape(B, HKV, D)

    in_maps = []
    for h in range(N_CORES):
        ktT = np.ascontiguousarray(kc[:, h, :].T)  # [128(d), 65536]
        khi, klo = _hi_lo(ktT)
        vna = np.ascontiguousarray(
            vcn[:, h, :].reshape(VCH, 128, D).transpose(1, 0, 2)
        )  # [128(p), 512(c), 128(d)]
        vhi, vlo = _hi_lo(vna)
        qh = np.ascontiguousarray(
            query.reshape(B, HKV, G, D)[:, h].transpose(2, 0, 1)
        )  # [128(d), 16(b), 4(g)]
        qhi, qlo = _hi_lo(qh)
        qcat = np.stack([qhi, qlo], axis=1)  # [128, 2, 16, 4]
        in_maps.append(
            {"khi": khi, "klo": klo, "vhi": vhi, "vlo": vlo, "qc": qcat}
        )

    lens = [int(x) for x in seq_lens]
    runs_key = tuple(
        (b, tuple(_coalesced_runs(block_tables[b], max(lens[b], 1))))
        for b in range(B)
    )
    cache_key = (tuple(lens), runs_key)
    if cache_key not in _CACHE:
        _CACHE[cache_key] = _build(lens, runs_key)
    nc = _CACHE[cache_key]

    kwargs = {}
    if TRACE:
        kwargs["trace"] = True
        kwargs["tmpdir"] = tempfile.mkdtemp(prefix="bass_attn_")
        if TRACE_ALL_CORES:
            kwargs["trace_cores"] = list(range(N_CORES))
    res = bass_utils.run_bass_kernel_spmd(
        nc, in_maps, list(range(N_CORES)), **kwargs
    )
    LAST_EXEC_NS = res.exec_time_ns
    LAST_RESULTS = res

    out = np.empty((B, H * D), dtype=np.float32)
    for h in range(N_CORES):
        out[:, h * G * 128 : (h + 1) * G * 128] = res.results[h]["out"]
    return out


# revision 10
# speedup vs baseline: 2.0858x; 1.2587x over previous
"""Paged-attention decode (vLLM-style) for Trainium2, 8 NeuronCores.

Sharding: tensor-parallel over KV heads. Core h owns KV head h and query
heads 4h..4h+3. block_tables / seq_lens are host-visible integers, so the
device program is fully static: gather addresses, masking boundaries and
loop trip counts are baked into the instruction stream at build time.

Precision strategy: TRN2's fp32 matmul is lowered by the compiler into two
half-rate passes with a ~210ns weight reload each, which made the fp32
version PE-bound (~430ns per logical matmul regardless of N). Instead we
split every fp32 operand x into bf16 hi + bf16 lo (x ~= hi + lo, the same
decomposition the hardware fp32 path uses), ship both halves from the host
(same total bytes as fp32), and run bf16 matmuls which get the fast weight
load path (~53ns per 128-col load). Products keep the hi*hi, hi*lo and
lo*hi terms (~2^-17 relative error, matching hardware fp32 matmul quality).
To avoid reloading the hi weights for the hi*lo term, the moving operand is
the concatenation [x_hi | x_lo]; the hi*lo product lands in spill columns
of PSUM and is folded in afterwards with one vector add per sequence.

Host prep (not on the device clock):
  - apply the 16-row new-token scatter to a host copy of the cache
    (exactly reference step 1),
  - K as K^T [D=128, 65536] hi/lo bf16 per head -> QK stationary tiles,
  - V as [p=128, chunk=512, d=128] hi/lo bf16 per head (pos = chunk*128+p)
    -> PV stationary tiles,
  - q as [128(d), 2(hi/lo), 16(b), 4(g)] bf16.

Device per sequence b (length L, C = ceil(L/128) chunks):
  scores   : per chunk c: matmul(psum[:, 8c:8c+8], lhsT=Khi_c, rhs=q_cat)
             + matmul(psum[:, 8c:8c+4], lhsT=Klo_c, rhs=q_hi, accumulate)
  probs    : DVE fold spill cols, ACT exp(scale*x), zero the padding tail
  p_cat    : DVE split probs into interleaved bf16 hi/lo [128, C, 2, 4]
  out[d,g] : per chunk c: matmul(acc[:, 0:8], lhsT=Vhi_c, rhs=p_cat_c)
             + matmul(acc[:, 0:4], lhsT=Vlo_c, rhs=p_hi_c), accumulated
  denom    : matmul(lhsT=ones[128,1] f32, rhs=probs f32) -> [1,4C],
             DVE strided reduce -> [1,4], DVE reciprocal -> r [1,4]
  bcast    : matmul(lhsT=ones[1,128], rhs=r) -> [128,4] (PSUM)
  final    : DVE fold PV spill, mul by r_bcast, PE transpose -> [4,128],
             copy to SBUF, DMA to out[b]
"""

import math
import os
import sys
import tempfile

import numpy as np

for _p in ("/opt/trn_rl_repo", "/opt/pypackages"):
    if os.path.isdir(_p) and _p not in sys.path:
        sys.path.append(_p)

import ml_dtypes

BF16 = ml_dtypes.bfloat16

B = 16
H = 32
HKV = 8
D = 128
G = H // HKV  # 4 query heads per kv head
BLOCK = 16
SLOTS = 65536  # total cache slots (NUM_BLOCKS * BLOCK)
VCH = SLOTS // 128  # 512 V chunks in the cache
SCALE = 1.0 / math.sqrt(D)
N_CORES = 8

# K/V slab DMAs are split into pieces of this many positions so compute can
# start before a whole sequence has landed.
DMA_SPLIT = 4096

TRACE = False
TRACE_ALL_CORES = False
LAST_EXEC_NS = None
LAST_RESULTS = None

_CACHE = {}


def _coalesced_runs(bt_row, L):
    """[(dst_pos, src_slot, n)] covering positions [0, L), merged when the
    source slots are contiguous (always, for arange block tables)."""
    runs = []
    nblk = (L + BLOCK - 1) // BLOCK
    for i in range(nblk):
        s = int(bt_row[i]) * BLOCK
        a = i * BLOCK
        n = min(BLOCK, L - a)
        if runs and runs[-1][1] + runs[-1][2] == s:
            runs[-1][2] += n
        else:
            runs.append([a, s, n])
    return [tuple(r) for r in runs]


def _v_rects(a, s, n):
    """Decompose a (dst_pos=a, src_slot=s, len=n) run into rectangles for the
    chunked V layout [p, c, d] with pos = c*128 + p. Yields
    (pd, ps, m, cd, cs, k): dst partitions [pd, pd+m) chunks [cd, cd+k),
    src partitions [ps, ps+m) chunks [cs, cs+k)."""
    rects = []
    if (s - a) % 128 == 0:
        dc = (s - a) // 128
        x = a
        end = a + n
        if x % 128:
            m = min(128 - x % 128, end - x)
            rects.append((x % 128, x % 128, m, x // 128, x // 128 + dc, 1))
            x += m
        k = (end - x) // 128
        if k > 0:
            rects.append((0, 0, 128, x // 128, x // 128 + dc, k))
            x += k * 128
        if x < end:
            rects.append((0, 0, end - x, x // 128, x // 128 + dc, 1))
    else:
        x = a
        end = a + n
        while x < end:
            y = x - a + s
            m = min(128 - x % 128, 128 - y % 128, end - x)
            rects.append((x % 128, y % 128, m, x // 128, y // 128, 1))
            x += m
    return rects


def _build(seq_lens, runs_key):
    import concourse.bass as bass  # noqa: F401
    import concourse.mybir as mybir
    import concourse.tile as tile
    from concourse import bacc
    from concourse.masks import make_identity

    f32 = mybir.dt.float32
    bf16 = mybir.dt.bfloat16
    Exp = mybir.ActivationFunctionType.Exp

    runs_per_seq = {b: runs for b, runs in runs_key}

    nc = bacc.Bacc(
        "TRN2", target_bir_lowering=False, debug=False, num_devices=N_CORES
    )
    kd = nc.dram_tensor("kd", [128, 2, SLOTS], bf16, kind="ExternalInput").ap()
    vd = nc.dram_tensor("vd", [128, 2, VCH, 128], bf16, kind="ExternalInput").ap()
    qc_d = nc.dram_tensor("qc", [128, 2, B, G], bf16, kind="ExternalInput").ap()
    outd = nc.dram_tensor("out", [B, G * 128], f32, kind="ExternalOutput").ap()
    out3 = outd.rearrange("b (g d) -> b g d", g=G)

    order = sorted(range(B), key=lambda b: -int(seq_lens[b]))

    with tile.TileContext(nc) as tc:
        with (
            tc.tile_pool(name="const", bufs=1) as const,
            tc.tile_pool(name="big", bufs=4) as big,
            tc.tile_pool(name="small", bufs=3) as small,
            tc.tile_pool(name="ps_sc", bufs=2, space="PSUM") as ps_sc,
            tc.tile_pool(name="ps_epi", bufs=2, space="PSUM") as ps_epi,
        ):
            qc_sb = const.tile([128, 2, B, G], bf16)
            nc.sync.dma_start(out=qc_sb, in_=qc_d)
            ones_col = const.tile([128, 1], f32)
            nc.vector.memset(ones_col, 1.0)
            ident = const.tile([128, 128], f32)
            make_identity(nc, ident)

            for b in order:
                L = int(seq_lens[b])
                C = (L + 127) // 128
                runs = runs_per_seq[b]

                # hi and lo planes side by side: kcat[:, j, :], vcat[:, j, ...]
                kcat = big.tile([128, 2, C * 128], bf16, tag="kcat", name=f"kc{b}")
                vcat = big.tile([128, 2, C, 128], bf16, tag="vcat", name=f"vc{b}")
                khi, klo = kcat[:, 0, :], kcat[:, 1, :]
                vhi, vlo = vcat[:, 0, :, :], vcat[:, 1, :, :]

                if L % 128:
                    # engine ops need 32-aligned partition bases, so zero the
                    # whole trailing chunk first; the DMAs then fill the valid
                    # rows (Tile orders the overlapping writes).
                    nc.vector.memset(vcat[:, :, C - 1, :], 0.0)
                for (a, s, n) in runs:
                    for off in range(0, n, DMA_SPLIT):
                        m = min(DMA_SPLIT, n - off)
                        nc.sync.dma_start(
                            out=kcat[:, :, a + off : a + off + m],
                            in_=kd[:, :, s + off : s + off + m],
                        )
                    for (pd, ps, m, cd, cs, k) in _v_rects(a, s, n):
                        for coff in range(0, k, DMA_SPLIT // 128):
                            kk = min(DMA_SPLIT // 128, k - coff)
                            nc.scalar.dma_start(
                                out=vcat[pd : pd + m, :, cd + coff : cd + coff + kk, :],
                                in_=vd[ps : ps + m, :, cs + coff : cs + coff + kk, :],
                            )
                if L < C * 128:
                    nc.vector.memset(kcat[:, :, L : C * 128], 0.0)

                # QK: psum cols per chunk: [8c, 8c+4) = hi*q_hi + lo*q_hi,
                #     [8c+4, 8c+8) = hi*q_lo spill
                scores = ps_sc.tile([128, 8 * C], f32, tag="scores", name=f"sc{b}")
                for c in range(C):
                    nc.tensor.matmul(
                        scores[:, 8 * c : 8 * c + 8],
                        lhsT=khi[:, c * 128 : (c + 1) * 128],
                        rhs=qc_sb[:, :, b, :],
                        start=True,
                        stop=False,
                        skip_group_check=True,
                    )
                    nc.tensor.matmul(
                        scores[:, 8 * c : 8 * c + 4],
                        lhsT=klo[:, c * 128 : (c + 1) * 128],
                        rhs=qc_sb[:, 0, b, :],
                        start=False,
                        stop=True,
                        skip_group_check=True,
                    )

                sc4 = scores.rearrange("p (c j g) -> p c j g", j=2, g=G)
                spill_s = small.tile([128, C, G], f32, tag="spill_s", name=f"ss{b}")
                nc.vector.tensor_copy(spill_s, sc4[:, :, 1, :])
                probs_f = small.tile([128, C, G], f32, tag="probs_f", name=f"pf{b}")
                nc.vector.tensor_add(probs_f, sc4[:, :, 0, :], spill_s)

                probs_e = small.tile([128, C * G], f32, tag="probs_e", name=f"pe{b}")
                pf2 = probs_f.rearrange("p c g -> p (c g)")
                if L % 128:
                    nc.vector.memset(probs_e[:, G * (C - 1) : G * C], 0.0)
                    if C > 1:
                        nc.scalar.activation(
                            probs_e[:, : G * (C - 1)],
                            pf2[:, : G * (C - 1)],
                            Exp,
                            scale=SCALE,
                        )
                    nc.scalar.activation(
                        probs_e[0 : L % 128, G * (C - 1) : G * C],
                        pf2[0 : L % 128, G * (C - 1) : G * C],
                        Exp,
                        scale=SCALE,
                    )
                else:
                    nc.scalar.activation(probs_e, pf2, Exp, scale=SCALE)

                # interleaved bf16 hi/lo of probs: [128, C, 2, G]
                pe3 = probs_e.rearrange("p (c g) -> p c g", g=G)
                pcat = small.tile([128, C, 2, G], bf16, tag="pcat", name=f"pc{b}")
                nc.vector.tensor_copy(pcat[:, :, 0, :], pe3)
                nc.vector.tensor_sub(pcat[:, :, 1, :], pe3, pcat[:, :, 0, :])

                # epi1: cols 0:4 = PV main, 4:8 = PV hi*p_lo spill,
                #       cols 8:8+4C (partition 0) = per-chunk prob sums
                epi1 = ps_epi.tile([128, 136], f32, tag="epi1", name=f"e1{b}")
                for c in range(C):
                    nc.tensor.matmul(
                        epi1[:, 0:8],
                        lhsT=vhi[:, c, :],
                        rhs=pcat[:, c, :, :],
                        start=(c == 0),
                        stop=False,
                        skip_group_check=True,
                    )
                    nc.tensor.matmul(
                        epi1[:, 0:4],
                        lhsT=vlo[:, c, :],
                        rhs=pcat[:, c, 0, :],
                        start=False,
                        stop=(c == C - 1),
                        skip_group_check=True,
                    )
                nc.tensor.matmul(
                    epi1[0:1, 8 : 8 + G * C],
                    lhsT=ones_col,
                    rhs=probs_e,
                    start=True,
                    stop=True,
                )

                den4 = small.tile([1, G], f32, tag="den4", name=f"d4{b}")
                nc.vector.reduce_sum(
                    out=den4,
                    in_=epi1[0:1, 8 : 8 + G * C].rearrange("p (c g) -> p g c", g=G),
                    axis=mybir.AxisListType.X,
                )

                pv_sp = small.tile([128, G], f32, tag="pv_sp", name=f"pv{b}")
                nc.vector.tensor_copy(pv_sp, epi1[:, 4:8])
                acc_sb = small.tile([128, G], f32, tag="acc_sb", name=f"ac{b}")
                nc.vector.tensor_add(acc_sb, epi1[:, 0:4], pv_sp)

                # epi2: col 0 = den^T [4,1]; cols 4:132 = acc^T [4,128]
                epi2 = ps_epi.tile([128, 132], f32, tag="epi2", name=f"e2{b}")
                nc.tensor.transpose(epi2[0:4, 0:1], den4, ident[0:1, 0:1])
                nc.tensor.transpose(epi2[0:4, 4:132], acc_sb, ident)
                r_t = small.tile([G, 1], f32, tag="r_t", name=f"rt{b}")
                nc.vector.reciprocal(r_t, epi2[0:4, 0:1])
                o_fin = small.tile([G, 128], f32, tag="o_fin", name=f"of{b}")
                nc.scalar.activation(
                    o_fin,
                    epi2[0:4, 4:132],
                    mybir.ActivationFunctionType.Copy,
                    scale=r_t,
                )
                nc.sync.dma_start(out=out3[b], in_=o_fin)

    nc.compile()
    return nc


def _hi_lo(x):
    hi = x.astype(BF16)
    lo = (x - hi.astype(np.float32)).astype(BF16)
    return hi, lo


def kernel(query, key, value, kv_cache, block_tables, seq_lens, slot_mapping):
    global LAST_EXEC_NS, LAST_RESULTS
    from concourse import bass_utils

    query = np.asarray(query, dtype=np.float32)
    key = np.asarray(key, dtype=np.float32)
    value = np.asarray(value, dtype=np.float32)
    kv_cache = np.asarray(kv_cache, dtype=np.float32)
    block_tables = np.asarray(block_tables)
    seq_lens = np.asarray(seq_lens)
    slot_mapping = np.asarray(slot_mapping)

    # --- host prep: apply new-token scatter (reference step 1) ---
    kc = np.array(kv_cache[0].reshape(SLOTS, HKV, D))
    vcn = np.array(kv_cache[1].reshape(SLOTS, HKV, D))
    kc[slot_mapping] = key.reshape(B, HKV, D)
    vcn[slot_mapping] = value.reshape(B, HKV, D)

    in_maps = []
    for h in range(N_CORES):
        ktT = np.ascontiguousarray(kc[:, h, :].T)  # [128(d), 65536]
        kcat = np.stack(_hi_lo(ktT), axis=1)  # [128, 2, 65536]
        vna = np.ascontiguousarray(
            vcn[:, h, :].reshape(VCH, 128, D).transpose(1, 0, 2)
        )  # [128(p), 512(c), 128(d)]
        vcat = np.stack(_hi_lo(vna), axis=1)  # [128, 2, 512, 128]
        qh = np.ascontiguousarray(
            query.reshape(B, HKV, G, D)[:, h].transpose(2, 0, 1)
        )  # [128(d), 16(b), 4(g)]
        qcat = np.stack(_hi_lo(qh), axis=1)  # [128, 2, 16, 4]
        in_maps.append({"kd": kcat, "vd": vcat, "qc": qcat})

    lens = [int(x) for x in seq_lens]
    runs_key = tuple(
        (b, tuple(_coalesced_runs(block_tables[b], max(lens[b], 1))))
        for b in range(B)
    )
    cache_key = (tuple(lens), runs_key)
    if cache_key not in _CACHE:
        _CACHE[cache_key] = _build(lens, runs_key)
    nc = _CACHE[cache_key]

    kwargs = {}
    if TRACE:
        kwargs["trace"] = True
        kwargs["tmpdir"] = tempfile.mkdtemp(prefix="bass_attn_")
        if TRACE_ALL_CORES:
            kwargs["trace_cores"] = list(range(N_CORES))
    res = bass_utils.run_bass_kernel_spmd(
        nc, in_maps, list(range(N_CORES)), **kwargs
    )
    LAST_EXEC_NS = res.exec_time_ns
    LAST_RESULTS = res

    out = np.empty((B, H * D), dtype=np.float32)
    for h in range(N_CORES):
        out[:, h * G * 128 : (h + 1) * G * 128] = res.results[h]["out"]
    return out
